# revision 3
# baseline (speedup 1.0000x reference)
"""NT-Xent instance loss on 8 Trainium2 NeuronCores, V4.

Symmetric 17-kiloblock decomposition: z (16384x128) is split into 16
kilobands; core c owns bands c and c+8 and computes each off-diagonal sim
kiloblock exactly once (17 column chunks of 1024 per core), contributing
row sums directly and column sums (the transposed block's row sums) via
ones-vector matmuls.

PSUM readout — the bottleneck; only ACT and DVE can read PSUM:
  - mode 'A' (ACT): exact exp activation, bf16 out, fp32 accum_out row
    sums into s_band.
  - mode 'D' (DVE): Schraudolph exp — uint16(sim*S + B) bitcast to bf16
    approximates exp(sim - C) to ~1.6% (the fp32->uint16 convert
    saturates negatives to 0 = bf16 +0.0; verified on HW). Row sums via
    bf16 accumulators: tensor_add on the otherwise-idle Pool engine
    (an SBUF-only engine), per-(band,m) fp32 tensor_reduce on DVE.

j==0 (the D=0 kiloblock, colsums discarded by the host) is all-D with a
-1e30 diagonal mask add on DVE (local, cheap); exact exp of the unmasked
diagonal would be inf, and unmasked Schraudolph bits can wrap into
large-negative bf16 for heavy rows.

Colsum PSUM: one [128,512] bank serves TWO j-chunks via tile_position
column groups (slots at partitions 0/32 and 64/96), so only one stage
copy + DMA per j-pair. Emission is software-pipelined: colsum matmuls
and accumulator adds trail their producer by DELAY m-iterations, the 8
colsum-free j==0 iterations are spliced between other chunks, and the
tail iterations flush fast so the drain stays short.
"""

import math

import numpy as np
import ml_dtypes

TRAIN_NUM = 8192
EMBED = 128
N = 2 * TRAIN_NUM
NCORES = 8
KB = 1024
M_PER_BAND = KB // 128
CHUNK = 1024                 # PSUM sim chunk (2 banks); 2 col-sum slots
NSLOTS = CHUNK // 512
ROWMAX_COEF = math.sqrt(2.0 * math.log(N - 1)) * math.sqrt(2.0)
C_SHIFT = 6.0

SCH_SCALE = 128.0 / math.log(2.0)
SCH_ADJ = -7.5

BANDS = [
    (0, 0, 0, 9),            # (band, row_base, col_base, n_chunks)
    (1, 8192, 8192, 8),
]
CHUNK_LIST = [(b, j) for b, _, _, nj in BANDS for j in range(nj)]
# colsum groups: pairs of j>=1 chunks sharing one PSUM bank
COL_GROUPS = []
for _b, _, _, _nj in BANDS:
    _js = [j for j in range(1, _nj)]
    for _i in range(0, len(_js), 2):
        COL_GROUPS.append((_b, tuple(_js[_i : _i + 2])))
N_COLCHUNKS = len(COL_GROUPS)  # 8 groups of up to 2 chunks


def _band_iter_order(n_chunks):
    """j-major for j>=1 with the 8 colsum-free j==0 iterations spliced in
    every 3rd slot near the front."""
    base = [(j, m) for j in range(1, n_chunks) for m in range(M_PER_BAND)]
    order = []
    k = 0
    for m in range(M_PER_BAND):
        order.extend(base[k : k + 2])
        k += 2
        order.append((0, m))
    order.extend(base[k:])
    return order


ITER_ORDER = []
for _b, _, _, _nj in BANDS:
    ITER_ORDER.extend((_b, j, m) for j, m in _band_iter_order(_nj))


def _gen_modes():
    """j0 all-D (diagonal masked there); elsewhere A/D Bresenham-spread.
    A84 / D52 overall."""
    n = len(ITER_ORDER)
    target = {"A": 84, "D": 52}
    credit = {e: 0.0 for e in "AD"}
    left = dict(target)
    assign = {}
    for i, (b, j, m) in enumerate(ITER_ORDER):
        for e in "AD":
            credit[e] += left[e] / (n - i)
        cands = [e for e in "AD" if left[e] > 0 and not (e == "A" and j == 0)]
        e = max(cands, key=lambda x: credit[x])
        assign[(b, j, m)] = e
        credit[e] -= 1.0
        left[e] -= 1
    return {
        (b, j): "".join(assign[(b, j, m)] for m in range(M_PER_BAND))
        for b, j in CHUNK_LIST
    }


MODES = _gen_modes()
DELAY = 8
TAIL_FAST = 8  # last iterations flush with DELAY=1

_cached = None


def _build():
    import concourse.bacc as bacc
    import concourse.tile as tile
    from concourse import mybir

    first_nonA = {}
    last_nonA = {}
    for b, j, m in ITER_ORDER:
        if MODES[(b, j)][m] != "A":
            first_nonA.setdefault((b, m), j)
            last_nonA[(b, m)] = j
    for b, _, _, nj in BANDS:
        for m in range(M_PER_BAND):
            assert (b, m) in first_nonA, f"(band={b}, m={m}) all A"

    # acc adds mostly on Pool; every 8th D chunk's add goes to DVE
    add_on_dve = set()
    di = 0
    for b, j, m in ITER_ORDER:
        if MODES[(b, j)][m] != "A":
            if di % 8 == 7:
                add_on_dve.add((b, j, m))
            di += 1

    # colsum group + column-group base per (band, j)
    cs_group = {}
    cs_base = {}
    cs_last_j = {}
    for gi, (b, js) in enumerate(COL_GROUPS):
        for sub, j in enumerate(js):
            cs_group[(b, j)] = gi
            cs_base[(b, j)] = 64 * sub
        cs_last_j[gi] = js[-1]

    n_iters = len(ITER_ORDER)

    nc = bacc.Bacc(
        "TRN2",
        target_bir_lowering=False,
        debug=False,
        num_devices=NCORES,
    )
    f32 = mybir.dt.float32
    bf16 = mybir.dt.bfloat16
    u16 = mybir.dt.uint16

    zT_dram = nc.dram_tensor("zT", (EMBED, N), bf16, kind="ExternalInput")
    cneg_dram = nc.dram_tensor("c_neg", (128, 2), f32, kind="ExternalInput")
    bsch_dram = nc.dram_tensor("b_sch", (128, 2), f32, kind="ExternalInput")
    s_dram = nc.dram_tensor("s_out", (128, 16), f32, kind="ExternalOutput")
    pos_dram = nc.dram_tensor("pos_out", (128, M_PER_BAND), f32, kind="ExternalOutput")
    col_dram = nc.dram_tensor(
        "col_out", (N_COLCHUNKS, 128, 512), f32, kind="ExternalOutput"
    )

    id_np = np.eye(128, dtype=np.float32)
    id_dram = nc.inline_tensor(id_np, name="id_mask")
    neg_np = np.zeros((128, 128), dtype=np.float32)
    np.fill_diagonal(neg_np, -1.0e30)
    neg_dram = nc.inline_tensor(neg_np, name="neg_mask")
    ones_dram = nc.inline_tensor(
        np.ones((128, 1), dtype=ml_dtypes.bfloat16), name="ones_vec"
    )

    with tile.TileContext(nc) as tc:
        with (
            tc.tile_pool(name="zbuf", bufs=1) as zpool,
            tc.tile_pool(name="consts", bufs=1) as cpool,
            tc.tile_pool(name="persist", bufs=1) as perpool,
            tc.tile_pool(name="psum", bufs=3, space="PSUM") as ppool,
            tc.tile_pool(name="colsum", bufs=2, space="PSUM") as cspool,
            tc.tile_pool(name="expout", bufs=12) as epool,
            tc.tile_pool(name="tmp", bufs=3) as tpool,
        ):
            z_sb = zpool.tile([EMBED, N], bf16)
            # two parallel queues for the first two slices the pipeline needs
            nc.sync.dma_start(out=z_sb[:, 0:CHUNK], in_=zT_dram[:, 0:CHUNK])
            nc.gpsimd.dma_start(
                out=z_sb[:, CHUNK : 2 * CHUNK], in_=zT_dram[:, CHUNK : 2 * CHUNK]
            )
            bsch_sb = cpool.tile([128, 2], f32)
            nc.sync.dma_start(out=bsch_sb, in_=bsch_dram[:, :])
            cneg_sb = cpool.tile([128, 2], f32)
            nc.sync.dma_start(out=cneg_sb, in_=cneg_dram[:, :])
            ones_sb = cpool.tile([128, 1], bf16)
            nc.sync.dma_start(out=ones_sb, in_=ones_dram[:, :])
            negm = cpool.tile([128, 128], f32)
            nc.sync.dma_start(out=negm, in_=neg_dram[:, :])
            idm = cpool.tile([128, 128], f32)
            nc.sync.dma_start(out=idm, in_=id_dram[:, :])

            warm = cpool.tile([128, 1], f32)
            nc.scalar.activation(
                out=warm,
                in_=cneg_sb[:, 0:1],
                func=mybir.ActivationFunctionType.Exp,
                bias=cneg_sb[:, 0:1],
                scale=0.0,
            )

            cuts = [2 * CHUNK, 6144, 10752, N]
            for qs, qe in zip(cuts, cuts[1:]):
                nc.sync.dma_start(out=z_sb[:, qs:qe], in_=zT_dram[:, qs:qe])

            s_band = perpool.tile([128, 16, 9], f32)
            nc.gpsimd.memset(s_band, 0.0)
            acc = perpool.tile([128, 16, CHUNK], bf16)
            acc_red = perpool.tile([128, 16], f32)
            s_red = perpool.tile([128, 16], f32)
            s_out_sb = perpool.tile([128, 16], f32)
            pos_sb = perpool.tile([128, M_PER_BAND], f32)

            pending = []

            def flush(now):
                rest = []
                for due, fn in pending:
                    if due <= now:
                        fn()
                    else:
                        rest.append((due, fn))
                pending[:] = rest

            def emit_bandA_final():
                nc.vector.tensor_reduce(
                    out=s_red[:, 0:8],
                    in_=s_band[:, 0:8, :],
                    axis=mybir.AxisListType.X,
                    op=mybir.AluOpType.add,
                )
                nc.vector.tensor_add(
                    s_out_sb[:, 0:8], s_red[:, 0:8], acc_red[:, 0:8]
                )
                nc.sync.dma_start(out=s_dram[:, 0:8], in_=s_out_sb[:, 0:8])
                nc.sync.dma_start(out=pos_dram[:, :], in_=pos_sb)

            it = 0
            for band, row_base, col_base, n_chunks in BANDS:
                if band == 1:
                    pending.append((it + DELAY + 2, emit_bandA_final))
                colsums = {}
                for j, m in _band_iter_order(n_chunks):
                    mode = MODES[(band, j)][m]
                    cbase = col_base + j * CHUNK
                    r0 = row_base + m * 128
                    k16 = band * 8 + m
                    delay = 1 if it >= n_iters - TAIL_FAST else DELAY
                    colsum = None
                    gi = cg = None
                    if j != 0:
                        gi = cs_group[(band, j)]
                        cg = cs_base[(band, j)]
                        if gi not in colsums:
                            colsums[gi] = cspool.tile(
                                [128, 512], f32, tag="cs", name=f"cs_{gi}"
                            )
                        colsum = colsums[gi]
                    ps = ppool.tile([128, CHUNK], f32, tag="ps")
                    for k in range(NSLOTS):
                        nc.tensor.matmul(
                            ps[:, k * 512 : (k + 1) * 512],
                            lhsT=z_sb[:, r0 : r0 + 128],
                            rhs=z_sb[:, cbase + k * 512 : cbase + (k + 1) * 512],
                            start=True,
                            stop=True,
                        )
                    assert not (j == 0 and mode == "A")
                    if j == 0:
                        # mask self-similarity diagonal (heavy rows would
                        # wrap the Schraudolph bits into large negatives)
                        nc.vector.tensor_add(
                            ps[:, m * 128 : m * 128 + 128],
                            ps[:, m * 128 : m * 128 + 128],
                            negm,
                        )
                    if band == 0 and j == 8:
                        tmp = tpool.tile([128, 128], f32, tag="pos")
                        nc.vector.tensor_mul(
                            tmp, ps[:, m * 128 : m * 128 + 128], idm
                        )
                        nc.vector.tensor_reduce(
                            out=pos_sb[:, m : m + 1],
                            in_=tmp,
                            axis=mybir.AxisListType.X,
                            op=mybir.AluOpType.add,
                        )
                    et = epool.tile(
                        [128, CHUNK], bf16, tag="exp", name=f"et_{band}_{j}_{m}"
                    )
                    if mode == "A":
                        nc.scalar.activation(
                            out=et,
                            in_=ps,
                            func=mybir.ActivationFunctionType.Exp,
                            bias=cneg_sb[:, band : band + 1],
                            scale=1.0,
                            accum_out=s_band[:, k16, j : j + 1],
                        )
                    else:
                        nc.vector.tensor_scalar(
                            out=et.bitcast(u16),
                            in0=ps,
                            scalar1=SCH_SCALE,
                            scalar2=bsch_sb[:, band : band + 1],
                            op0=mybir.AluOpType.mult,
                            op1=mybir.AluOpType.add,
                        )

                        def emit_acc(et=et, k16=k16, band=band, m=m, j=j):
                            eng = (
                                nc.vector
                                if (band, j, m) in add_on_dve
                                else nc.gpsimd
                            )
                            if first_nonA[(band, m)] == j:
                                eng.tensor_copy(acc[:, k16, :], et)
                            else:
                                eng.tensor_add(acc[:, k16, :], acc[:, k16, :], et)
                            if last_nonA[(band, m)] == j:
                                nc.vector.tensor_reduce(
                                    out=acc_red[:, k16 : k16 + 1],
                                    in_=acc[:, k16, :],
                                    axis=mybir.AxisListType.X,
                                    op=mybir.AluOpType.add,
                                )

                        pending.append((it + delay, emit_acc))

                    if j != 0:
                        def emit_cs(et=et, colsum=colsum, m=m, cg=cg):
                            for s in range(NSLOTS):
                                nc.tensor.matmul(
                                    colsum[cg + 32 * s : cg + 32 * s + 1, :],
                                    lhsT=ones_sb,
                                    rhs=et[:, s * 512 : (s + 1) * 512],
                                    start=(m == 0),
                                    stop=(m == M_PER_BAND - 1),
                                    tile_position=(0, cg + 32 * s),
                                )

                        pending.append((it + delay, emit_cs))

                        if m == M_PER_BAND - 1 and cs_last_j[gi] == j:
                            def emit_stage(colsum=colsum, gi=gi):
                                stage = tpool.tile(
                                    [128, 512], f32, tag="stage"
                                )
                                nc.vector.tensor_copy(stage, colsum)
                                nc.sync.dma_start(
                                    out=col_dram[gi, :, :], in_=stage
                                )

                            pending.append((it + delay + 1, emit_stage))

                    it += 1
                    flush(it)

            flush(it + DELAY + 10)

            nc.vector.tensor_reduce(
                out=s_red[:, 8:16],
                in_=s_band[:, 8:16, :],
                axis=mybir.AxisListType.X,
                op=mybir.AluOpType.add,
            )
            nc.vector.tensor_add(
                s_out_sb[:, 8:16], s_red[:, 8:16], acc_red[:, 8:16]
            )
            nc.sync.dma_start(out=s_dram[:, 8:16], in_=s_out_sb[:, 8:16])

    nc.compile()
    return nc


def _get_nc():
    global _cached
    if _cached is None:
        _cached = _build()
    return _cached


def _prep(z_i: np.ndarray, z_j: np.ndarray):
    z = np.concatenate(
        [np.asarray(z_i, np.float32), np.asarray(z_j, np.float32)], axis=0
    )
    w = z * np.float32(math.sqrt(2.0))
    wnorm = np.linalg.norm(w.astype(np.float64), axis=1)
    c_band = np.array(
        [
            ROWMAX_COEF * np.median(wnorm[b * KB : (b + 1) * KB]) + C_SHIFT
            for b in range(16)
        ],
        dtype=np.float64,
    )
    in_maps = []
    for c in range(NCORES):
        wc = np.roll(w, -c * KB, axis=0)
        zT = np.ascontiguousarray(wc.T).astype(ml_dtypes.bfloat16)
        cneg = np.zeros((128, 2), dtype=np.float32)
        cneg[:, 0] = -c_band[c]
        cneg[:, 1] = -c_band[c + 8]
        bsch = np.zeros((128, 2), dtype=np.float32)
        bsch[:, 0] = 16256.0 - c_band[c] * SCH_SCALE + SCH_ADJ
        bsch[:, 1] = 16256.0 - c_band[c + 8] * SCH_SCALE + SCH_ADJ
        in_maps.append({"zT": zT, "c_neg": cneg, "b_sch": bsch})
    return w, c_band, in_maps


def _finish(w, c_band, results):
    s_abs = np.zeros(N, dtype=np.float64)
    pos = np.zeros(N, dtype=np.float64)
    for c in range(NCORES):
        r = results[c]
        s_dev = r["s_out"].astype(np.float64)
        pos_dev = r["pos_out"].astype(np.float64)
        col_dev = r["col_out"].astype(np.float64)
        for band, kb in ((0, c), (1, c + 8)):
            scale = math.exp(c_band[kb])
            rows = s_dev[:, band * 8 : band * 8 + 8].T.reshape(KB)
            g0 = kb * KB
            s_abs[g0 : g0 + KB] += rows * scale
        for gi, (band, js) in enumerate(COL_GROUPS):
            kb = c if band == 0 else c + 8
            scale = math.exp(c_band[kb])
            for sub, j in enumerate(js):
                for sl in range(NSLOTS):
                    L = band * 8192 + j * CHUNK + sl * 512
                    vals = col_dev[gi, 64 * sub + 32 * sl, :] * scale
                    g = (c * KB + L) % N
                    s_abs[g : g + 512] += vals
        p_rows = pos_dev.T.reshape(KB)
        pos[c * KB : c * KB + KB] = p_rows
        pos[c * KB + 8192 : c * KB + 8192 + KB] = p_rows

    with np.errstate(divide="ignore", invalid="ignore"):
        lse = np.log(s_abs)
    bad = ~np.isfinite(lse)
    if bad.any():
        idx = np.nonzero(bad)[0]
        wb = w[idx].astype(np.float64)
        sim_b = wb @ w.astype(np.float64).T
        for ii, rr in enumerate(idx):
            sim_b[ii, rr] = -np.inf
        m_b = sim_b.max(axis=1)
        lse[idx] = np.log(np.exp(sim_b - m_b[:, None]).sum(axis=1)) + m_b
        pos_idx = np.where(idx < TRAIN_NUM, idx + TRAIN_NUM, idx - TRAIN_NUM)
        pos[idx] = np.einsum("ij,ij->i", wb, w[pos_idx].astype(np.float64))
    loss = (lse - pos).mean()
    return np.float32(loss)


def run(z_i, z_j, trace=False, **kw):
    from concourse.bass_utils import run_bass_kernel_spmd

    nc = _get_nc()
    w, c_band, in_maps = _prep(z_i, z_j)
    res = run_bass_kernel_spmd(
        nc, in_maps, core_ids=list(range(NCORES)), trace=trace, **kw
    )
    return _finish(w, c_band, res.results), res


def kernel(z_i, z_j):
    loss, _ = run(z_i, z_j, trace=False)
    return loss


# revision 4
# speedup vs baseline: 1.0267x; 1.0267x over previous
"""NT-Xent instance loss on 8 Trainium2 NeuronCores, V4.

Symmetric 17-kiloblock decomposition: z (16384x128) is split into 16
kilobands; core c owns bands c and c+8 and computes each off-diagonal sim
kiloblock exactly once (17 column chunks of 1024 per core), contributing
row sums directly and column sums (the transposed block's row sums) via
ones-vector matmuls.

PSUM readout — the bottleneck; only ACT and DVE can read PSUM:
  - mode 'A' (ACT): exact exp activation, bf16 out, fp32 accum_out row
    sums into s_band.
  - mode 'D' (DVE): Schraudolph exp — uint16(sim*S + B) bitcast to bf16
    approximates exp(sim - C) to ~1.6% (the fp32->uint16 convert
    saturates negatives to 0 = bf16 +0.0; verified on HW). Row sums via
    bf16 accumulators: tensor_add on the otherwise-idle Pool engine
    (an SBUF-only engine), per-(band,m) fp32 tensor_reduce on DVE.

j==0 (the D=0 kiloblock, colsums discarded by the host) is all-D with a
-1e30 diagonal mask add on DVE (local, cheap); exact exp of the unmasked
diagonal would be inf, and unmasked Schraudolph bits can wrap into
large-negative bf16 for heavy rows.

Colsum PSUM: one [128,512] bank serves TWO j-chunks via tile_position
column groups (slots at partitions 0/32 and 64/96), so only one stage
copy + DMA per j-pair. Emission is software-pipelined: colsum matmuls
and accumulator adds trail their producer by DELAY m-iterations, the 8
colsum-free j==0 iterations are spliced between other chunks, and the
tail iterations flush fast so the drain stays short.
"""

import math

import numpy as np
import ml_dtypes

TRAIN_NUM = 8192
EMBED = 128
N = 2 * TRAIN_NUM
NCORES = 8
KB = 1024
M_PER_BAND = KB // 128
CHUNK = 1024                 # PSUM sim chunk (2 banks); 2 col-sum slots
NSLOTS = CHUNK // 512
ROWMAX_COEF = math.sqrt(2.0 * math.log(N - 1)) * math.sqrt(2.0)
C_SHIFT = 6.0

SCH_SCALE = 128.0 / math.log(2.0)
SCH_ADJ = -7.5

BANDS = [
    (0, 0, 0, 9),            # (band, row_base, col_base, n_chunks)
    (1, 8192, 8192, 8),
]
CHUNK_LIST = [(b, j) for b, _, _, nj in BANDS for j in range(nj)]
# colsum groups: pairs of j>=1 chunks sharing one PSUM bank
COL_GROUPS = []
for _b, _, _, _nj in BANDS:
    _js = [j for j in range(1, _nj)]
    for _i in range(0, len(_js), 2):
        COL_GROUPS.append((_b, tuple(_js[_i : _i + 2])))
N_COLCHUNKS = len(COL_GROUPS)  # 8 groups of up to 2 chunks


def _band_iter_order(n_chunks):
    """j-major for j>=1 with the 8 colsum-free j==0 iterations spliced in
    every 3rd slot near the front."""
    base = [(j, m) for j in range(1, n_chunks) for m in range(M_PER_BAND)]
    order = []
    k = 0
    for m in range(M_PER_BAND):
        order.extend(base[k : k + 2])
        k += 2
        order.append((0, m))
    order.extend(base[k:])
    return order


ITER_ORDER = []
for _b, _, _, _nj in BANDS:
    ITER_ORDER.extend((_b, j, m) for j, m in _band_iter_order(_nj))


def _gen_modes():
    """j0 all-D (diagonal masked there); elsewhere A/D Bresenham-spread.
    A84 / D52 overall."""
    n = len(ITER_ORDER)
    target = {"A": 82, "D": 54}
    credit = {e: 0.0 for e in "AD"}
    left = dict(target)
    assign = {}
    prev = ""
    for i, (b, j, m) in enumerate(ITER_ORDER):
        for e in "AD":
            credit[e] += left[e] / (n - i)
        cands = [e for e in "AD" if left[e] > 0 and not (e == "A" and j == 0)]
        e = max(cands, key=lambda x: credit[x])
        assign[(b, j, m)] = e
        credit[e] -= 1.0
        left[e] -= 1
    # re-lay the last 16 positions so A's alternate with D's (the adaptive
    # spread tends to leave an A-convoy at the very end, stalling PE there)
    tail_keys = ITER_ORDER[n - 16 :]
    tail_modes = [assign[k] for k in tail_keys]
    n_a = tail_modes.count("A")
    n_d = len(tail_modes) - n_a
    lay = []
    a_left, d_left = n_a, n_d
    take_a = True
    for _ in range(len(tail_modes)):
        if (take_a and a_left > 0) or d_left == 0:
            lay.append("A"); a_left -= 1
        else:
            lay.append("D"); d_left -= 1
        take_a = not take_a
    for k, e in zip(tail_keys, lay):
        assign[k] = e
    return {
        (b, j): "".join(assign[(b, j, m)] for m in range(M_PER_BAND))
        for b, j in CHUNK_LIST
    }


MODES = _gen_modes()
DELAY = 8
TAIL_FAST = 8  # last iterations flush with DELAY=1

_cached = None


def _build():
    import concourse.bacc as bacc
    import concourse.tile as tile
    from concourse import mybir

    first_nonA = {}
    last_nonA = {}
    for b, j, m in ITER_ORDER:
        if MODES[(b, j)][m] != "A":
            first_nonA.setdefault((b, m), j)
            last_nonA[(b, m)] = j
    for b, _, _, nj in BANDS:
        for m in range(M_PER_BAND):
            assert (b, m) in first_nonA, f"(band={b}, m={m}) all A"

    # acc adds mostly on Pool; every 8th D chunk's add goes to DVE
    add_on_dve = set()
    di = 0
    for b, j, m in ITER_ORDER:
        if MODES[(b, j)][m] != "A":
            if di % 8 == 7:
                add_on_dve.add((b, j, m))
            di += 1

    # colsum group + column-group base per (band, j)
    cs_group = {}
    cs_base = {}
    cs_last_j = {}
    for gi, (b, js) in enumerate(COL_GROUPS):
        for sub, j in enumerate(js):
            cs_group[(b, j)] = gi
            cs_base[(b, j)] = 64 * sub
        cs_last_j[gi] = js[-1]

    n_iters = len(ITER_ORDER)

    nc = bacc.Bacc(
        "TRN2",
        target_bir_lowering=False,
        debug=False,
        num_devices=NCORES,
    )
    f32 = mybir.dt.float32
    bf16 = mybir.dt.bfloat16
    u16 = mybir.dt.uint16

    zT_dram = nc.dram_tensor("zT", (EMBED, N), bf16, kind="ExternalInput")
    cneg_dram = nc.dram_tensor("c_neg", (128, 2), f32, kind="ExternalInput")
    bsch_dram = nc.dram_tensor("b_sch", (128, 2), f32, kind="ExternalInput")
    s_dram = nc.dram_tensor("s_out", (128, 16), f32, kind="ExternalOutput")
    pos_dram = nc.dram_tensor("pos_out", (128, M_PER_BAND), f32, kind="ExternalOutput")
    col_dram = nc.dram_tensor(
        "col_out", (N_COLCHUNKS, 128, 512), f32, kind="ExternalOutput"
    )

    id_np = np.eye(128, dtype=np.float32)
    id_dram = nc.inline_tensor(id_np, name="id_mask")
    neg_np = np.zeros((128, 128), dtype=np.float32)
    np.fill_diagonal(neg_np, -1.0e30)
    neg_dram = nc.inline_tensor(neg_np, name="neg_mask")
    ones_dram = nc.inline_tensor(
        np.ones((128, 1), dtype=ml_dtypes.bfloat16), name="ones_vec"
    )

    with tile.TileContext(nc) as tc:
        with (
            tc.tile_pool(name="zbuf", bufs=1) as zpool,
            tc.tile_pool(name="consts", bufs=1) as cpool,
            tc.tile_pool(name="persist", bufs=1) as perpool,
            tc.tile_pool(name="psum", bufs=3, space="PSUM") as ppool,
            tc.tile_pool(name="colsum", bufs=2, space="PSUM") as cspool,
            tc.tile_pool(name="expout", bufs=12) as epool,
            tc.tile_pool(name="tmp", bufs=3) as tpool,
        ):
            z_sb = zpool.tile([EMBED, N], bf16)
            # exp ACT-table load first — needs no DMA'd inputs
            warm_in = cpool.tile([128, 1], f32)
            nc.gpsimd.memset(warm_in, 0.0)
            warm = cpool.tile([128, 1], f32)
            nc.scalar.activation(
                out=warm,
                in_=warm_in,
                func=mybir.ActivationFunctionType.Exp,
                bias=0.0,
                scale=0.0,
            )
            # two parallel queues for the first two slices the pipeline needs
            nc.sync.dma_start(out=z_sb[:, 0:CHUNK], in_=zT_dram[:, 0:CHUNK])
            nc.gpsimd.dma_start(
                out=z_sb[:, CHUNK : 2 * CHUNK], in_=zT_dram[:, CHUNK : 2 * CHUNK]
            )
            cneg_sb = cpool.tile([128, 2], f32)
            nc.sync.dma_start(out=cneg_sb, in_=cneg_dram[:, :])
            bsch_sb = cpool.tile([128, 2], f32)
            nc.sync.dma_start(out=bsch_sb, in_=bsch_dram[:, :])
            ones_sb = cpool.tile([128, 1], bf16)
            nc.sync.dma_start(out=ones_sb, in_=ones_dram[:, :])
            negm = cpool.tile([128, 128], f32)
            nc.gpsimd.dma_start(out=negm, in_=neg_dram[:, :])
            idm = cpool.tile([128, 128], f32)
            nc.sync.dma_start(out=idm, in_=id_dram[:, :])

            cuts = [2 * CHUNK, 6144, 10752, N]
            for qs, qe in zip(cuts, cuts[1:]):
                nc.sync.dma_start(out=z_sb[:, qs:qe], in_=zT_dram[:, qs:qe])

            s_band = perpool.tile([128, 16, 9], f32)
            nc.gpsimd.memset(s_band, 0.0)
            acc = perpool.tile([128, 16, CHUNK], bf16)
            acc_red = perpool.tile([128, 16], f32)
            s_red = perpool.tile([128, 16], f32)
            s_out_sb = perpool.tile([128, 16], f32)
            pos_sb = perpool.tile([128, M_PER_BAND], f32)

            pending = []       # colsum / acc / final emissions
            pending_dve = []   # DVE-heavy deferred ops (reduces): 1 per iter

            def flush(now, dve_budget=1):
                rest = []
                for due, fn in pending:
                    if due <= now:
                        fn()
                    else:
                        rest.append((due, fn))
                pending[:] = rest
                n = 0
                rest = []
                for due, fn in pending_dve:
                    if due <= now and n < dve_budget:
                        fn()
                        n += 1
                    else:
                        rest.append((due, fn))
                pending_dve[:] = rest

            def emit_bandA_final():
                nc.vector.tensor_reduce(
                    out=s_red[:, 0:8],
                    in_=s_band[:, 0:8, :],
                    axis=mybir.AxisListType.X,
                    op=mybir.AluOpType.add,
                )
                nc.vector.tensor_add(
                    s_out_sb[:, 0:8], s_red[:, 0:8], acc_red[:, 0:8]
                )
                nc.sync.dma_start(out=s_dram[:, 0:8], in_=s_out_sb[:, 0:8])
                nc.sync.dma_start(out=pos_dram[:, :], in_=pos_sb)

            it = 0
            for band, row_base, col_base, n_chunks in BANDS:
                if band == 1:
                    pending.append((it + DELAY + 2, emit_bandA_final))
                colsums = {}
                for j, m in _band_iter_order(n_chunks):
                    mode = MODES[(band, j)][m]
                    cbase = col_base + j * CHUNK
                    r0 = row_base + m * 128
                    k16 = band * 8 + m
                    delay = 1 if it >= n_iters - TAIL_FAST else DELAY
                    colsum = None
                    gi = cg = None
                    if j != 0:
                        gi = cs_group[(band, j)]
                        cg = cs_base[(band, j)]
                        if gi not in colsums:
                            colsums[gi] = cspool.tile(
                                [128, 512], f32, tag="cs", name=f"cs_{gi}"
                            )
                        colsum = colsums[gi]
                    ps = ppool.tile([128, CHUNK], f32, tag="ps")
                    for k in range(NSLOTS):
                        nc.tensor.matmul(
                            ps[:, k * 512 : (k + 1) * 512],
                            lhsT=z_sb[:, r0 : r0 + 128],
                            rhs=z_sb[:, cbase + k * 512 : cbase + (k + 1) * 512],
                            start=True,
                            stop=True,
                        )
                    assert not (j == 0 and mode == "A")
                    if j == 0:
                        # mask self-similarity diagonal (heavy rows would
                        # wrap the Schraudolph bits into large negatives)
                        nc.vector.tensor_add(
                            ps[:, m * 128 : m * 128 + 128],
                            ps[:, m * 128 : m * 128 + 128],
                            negm,
                        )
                    if band == 0 and j == 8:
                        tmp = tpool.tile([128, 128], f32, tag="pos")
                        nc.vector.tensor_mul(
                            tmp, ps[:, m * 128 : m * 128 + 128], idm
                        )
                        nc.vector.tensor_reduce(
                            out=pos_sb[:, m : m + 1],
                            in_=tmp,
                            axis=mybir.AxisListType.X,
                            op=mybir.AluOpType.add,
                        )
                    et = epool.tile(
                        [128, CHUNK], bf16, tag="exp", name=f"et_{band}_{j}_{m}"
                    )
                    if mode == "A":
                        nc.scalar.activation(
                            out=et,
                            in_=ps,
                            func=mybir.ActivationFunctionType.Exp,
                            bias=cneg_sb[:, band : band + 1],
                            scale=1.0,
                            accum_out=s_band[:, k16, j : j + 1],
                        )
                    else:
                        nc.vector.tensor_scalar(
                            out=et.bitcast(u16),
                            in0=ps,
                            scalar1=SCH_SCALE,
                            scalar2=bsch_sb[:, band : band + 1],
                            op0=mybir.AluOpType.mult,
                            op1=mybir.AluOpType.add,
                        )

                        def emit_acc(et=et, k16=k16, band=band, m=m, j=j):
                            eng = (
                                nc.vector
                                if (band, j, m) in add_on_dve
                                else nc.gpsimd
                            )
                            if first_nonA[(band, m)] == j:
                                eng.tensor_copy(acc[:, k16, :], et)
                            else:
                                eng.tensor_add(acc[:, k16, :], acc[:, k16, :], et)

                        pending.append((it + delay, emit_acc))
                        if last_nonA[(band, m)] == j:
                            def emit_red(k16=k16):
                                nc.vector.tensor_reduce(
                                    out=acc_red[:, k16 : k16 + 1],
                                    in_=acc[:, k16, :],
                                    axis=mybir.AxisListType.X,
                                    op=mybir.AluOpType.add,
                                )

                            pending_dve.append((it + delay + 1, emit_red))

                    if j != 0:
                        def emit_cs(et=et, colsum=colsum, m=m, cg=cg):
                            for s in range(NSLOTS):
                                nc.tensor.matmul(
                                    colsum[cg + 32 * s : cg + 32 * s + 1, :],
                                    lhsT=ones_sb,
                                    rhs=et[:, s * 512 : (s + 1) * 512],
                                    start=(m == 0),
                                    stop=(m == M_PER_BAND - 1),
                                    tile_position=(0, cg + 32 * s),
                                )

                        pending.append((it + delay, emit_cs))

                        if m == M_PER_BAND - 1 and cs_last_j[gi] == j:
                            def emit_stage(colsum=colsum, gi=gi):
                                stage = tpool.tile(
                                    [128, 512], f32, tag="stage"
                                )
                                nc.scalar.copy(stage, colsum)
                                nc.sync.dma_start(
                                    out=col_dram[gi, :, :], in_=stage
                                )

                            pending.append((it + delay + 1, emit_stage))

                    it += 1
                    flush(it)

            flush(it + DELAY + 10, dve_budget=99)

            nc.vector.tensor_reduce(
                out=s_red[:, 8:16],
                in_=s_band[:, 8:16, :],
                axis=mybir.AxisListType.X,
                op=mybir.AluOpType.add,
            )
            nc.vector.tensor_add(
                s_out_sb[:, 8:16], s_red[:, 8:16], acc_red[:, 8:16]
            )
            nc.sync.dma_start(out=s_dram[:, 8:16], in_=s_out_sb[:, 8:16])

    nc.compile()
    return nc


def _get_nc():
    global _cached
    if _cached is None:
        _cached = _build()
    return _cached


def _prep(z_i: np.ndarray, z_j: np.ndarray):
    z = np.concatenate(
        [np.asarray(z_i, np.float32), np.asarray(z_j, np.float32)], axis=0
    )
    w = z * np.float32(math.sqrt(2.0))
    wnorm = np.linalg.norm(w.astype(np.float64), axis=1)
    c_band = np.array(
        [
            ROWMAX_COEF * np.median(wnorm[b * KB : (b + 1) * KB]) + C_SHIFT
            for b in range(16)
        ],
        dtype=np.float64,
    )
    in_maps = []
    for c in range(NCORES):
        wc = np.roll(w, -c * KB, axis=0)
        zT = np.ascontiguousarray(wc.T).astype(ml_dtypes.bfloat16)
        cneg = np.zeros((128, 2), dtype=np.float32)
        cneg[:, 0] = -c_band[c]
        cneg[:, 1] = -c_band[c + 8]
        bsch = np.zeros((128, 2), dtype=np.float32)
        bsch[:, 0] = 16256.0 - c_band[c] * SCH_SCALE + SCH_ADJ
        bsch[:, 1] = 16256.0 - c_band[c + 8] * SCH_SCALE + SCH_ADJ
        in_maps.append({"zT": zT, "c_neg": cneg, "b_sch": bsch})
    return w, c_band, in_maps


def _finish(w, c_band, results):
    s_abs = np.zeros(N, dtype=np.float64)
    pos = np.zeros(N, dtype=np.float64)
    for c in range(NCORES):
        r = results[c]
        s_dev = r["s_out"].astype(np.float64)
        pos_dev = r["pos_out"].astype(np.float64)
        col_dev = r["col_out"].astype(np.float64)
        for band, kb in ((0, c), (1, c + 8)):
            scale = math.exp(c_band[kb])
            rows = s_dev[:, band * 8 : band * 8 + 8].T.reshape(KB)
            g0 = kb * KB
            s_abs[g0 : g0 + KB] += rows * scale
        for gi, (band, js) in enumerate(COL_GROUPS):
            kb = c if band == 0 else c + 8
            scale = math.exp(c_band[kb])
            for sub, j in enumerate(js):
                for sl in range(NSLOTS):
                    L = band * 8192 + j * CHUNK + sl * 512
                    vals = col_dev[gi, 64 * sub + 32 * sl, :] * scale
                    g = (c * KB + L) % N
                    s_abs[g : g + 512] += vals
        p_rows = pos_dev.T.reshape(KB)
        pos[c * KB : c * KB + KB] = p_rows
        pos[c * KB + 8192 : c * KB + 8192 + KB] = p_rows

    with np.errstate(divide="ignore", invalid="ignore"):
        lse = np.log(s_abs)
    bad = ~np.isfinite(lse)
    if bad.any():
        idx = np.nonzero(bad)[0]
        wb = w[idx].astype(np.float64)
        sim_b = wb @ w.astype(np.float64).T
        for ii, rr in enumerate(idx):
            sim_b[ii, rr] = -np.inf
        m_b = sim_b.max(axis=1)
        lse[idx] = np.log(np.exp(sim_b - m_b[:, None]).sum(axis=1)) + m_b
        pos_idx = np.where(idx < TRAIN_NUM, idx + TRAIN_NUM, idx - TRAIN_NUM)
        pos[idx] = np.einsum("ij,ij->i", wb, w[pos_idx].astype(np.float64))
    loss = (lse - pos).mean()
    return np.float32(loss)


def run(z_i, z_j, trace=False, **kw):
    from concourse.bass_utils import run_bass_kernel_spmd

    nc = _get_nc()
    w, c_band, in_maps = _prep(z_i, z_j)
    res = run_bass_kernel_spmd(
        nc, in_maps, core_ids=list(range(NCORES)), trace=trace, **kw
    )
    return _finish(w, c_band, res.results), res


def kernel(z_i, z_j):
    loss, _ = run(z_i, z_j, trace=False)
    return loss


# revision 5
# speedup vs baseline: 1.0782x; 1.0502x over previous
"""NT-Xent instance loss on 8 Trainium2 NeuronCores, V4.

Symmetric 17-kiloblock decomposition: z (16384x128) is split into 16
kilobands; core c owns bands c and c+8 and computes each off-diagonal sim
kiloblock exactly once (17 column chunks of 1024 per core), contributing
row sums directly and column sums (the transposed block's row sums) via
ones-vector matmuls.

PSUM readout — the bottleneck; only ACT and DVE can read PSUM:
  - mode 'A' (ACT): exact exp activation, bf16 out, fp32 accum_out row
    sums into s_band.
  - mode 'D' (DVE): Schraudolph exp — uint16(sim*S + B) bitcast to bf16
    approximates exp(sim - C) to ~1.6% (the fp32->uint16 convert
    saturates negatives to 0 = bf16 +0.0; verified on HW). Row sums via
    bf16 accumulators: tensor_add on the otherwise-idle Pool engine
    (an SBUF-only engine), per-(band,m) fp32 tensor_reduce on DVE.

j==0 (the D=0 kiloblock, colsums discarded by the host) is all-D with a
-1e30 diagonal mask add on DVE (local, cheap); exact exp of the unmasked
diagonal would be inf, and unmasked Schraudolph bits can wrap into
large-negative bf16 for heavy rows.

Colsum PSUM: one [128,512] bank serves TWO j-chunks via tile_position
column groups (slots at partitions 0/32 and 64/96), so only one stage
copy + DMA per j-pair. Emission is software-pipelined: colsum matmuls
and accumulator adds trail their producer by DELAY m-iterations, the 8
colsum-free j==0 iterations are spliced between other chunks, and the
tail iterations flush fast so the drain stays short.
"""

import math

import numpy as np
import ml_dtypes

TRAIN_NUM = 8192
EMBED = 128
N = 2 * TRAIN_NUM
NCORES = 8
KB = 1024
M_PER_BAND = KB // 128
CHUNK = 1024                 # PSUM sim chunk (2 banks); 2 col-sum slots
NSLOTS = CHUNK // 512
ROWMAX_COEF = math.sqrt(2.0 * math.log(N - 1)) * math.sqrt(2.0)
C_SHIFT = 6.0

SCH_SCALE = 128.0 / math.log(2.0)
SCH_ADJ = -7.5

BANDS = [
    (0, 0, 0, 9),            # (band, row_base, col_base, n_chunks)
    (1, 8192, 8192, 8),
]
CHUNK_LIST = [(b, j) for b, _, _, nj in BANDS for j in range(nj)]
# colsum groups: pairs of j>=1 chunks sharing one PSUM bank
COL_GROUPS = []
for _b, _, _, _nj in BANDS:
    _js = [j for j in range(1, _nj)]
    for _i in range(0, len(_js), 2):
        COL_GROUPS.append((_b, tuple(_js[_i : _i + 2])))
N_COLCHUNKS = len(COL_GROUPS)  # 8 groups of up to 2 chunks


def _band_iter_order(n_chunks):
    """j-major for j>=1 with the 8 colsum-free j==0 iterations spliced in
    every 3rd slot near the front."""
    base = [(j, m) for j in range(1, n_chunks) for m in range(M_PER_BAND)]
    order = []
    k = 0
    for m in range(M_PER_BAND):
        order.extend(base[k : k + 2])
        k += 2
        order.append((0, m))
    order.extend(base[k:])
    return order


ITER_ORDER = []
for _b, _, _, _nj in BANDS:
    ITER_ORDER.extend((_b, j, m) for j, m in _band_iter_order(_nj))


def _gen_modes():
    """j0 all-D (diagonal masked there); elsewhere A/D Bresenham-spread.
    A84 / D52 overall."""
    n = len(ITER_ORDER)
    target = {"A": 82, "D": 54}
    credit = {e: 0.0 for e in "AD"}
    left = dict(target)
    assign = {}
    prev = ""
    for i, (b, j, m) in enumerate(ITER_ORDER):
        for e in "AD":
            credit[e] += left[e] / (n - i)
        cands = [e for e in "AD" if left[e] > 0 and not (e == "A" and j == 0)]
        e = max(cands, key=lambda x: credit[x])
        assign[(b, j, m)] = e
        credit[e] -= 1.0
        left[e] -= 1
    # re-lay the last 16 positions so A's alternate with D's (the adaptive
    # spread tends to leave an A-convoy at the very end, stalling PE there)
    tail_keys = ITER_ORDER[n - 16 :]
    tail_modes = [assign[k] for k in tail_keys]
    n_a = tail_modes.count("A")
    n_d = len(tail_modes) - n_a
    lay = []
    a_left, d_left = n_a, n_d
    take_a = True
    for _ in range(len(tail_modes)):
        if (take_a and a_left > 0) or d_left == 0:
            lay.append("A"); a_left -= 1
        else:
            lay.append("D"); d_left -= 1
        take_a = not take_a
    for k, e in zip(tail_keys, lay):
        assign[k] = e
    return {
        (b, j): "".join(assign[(b, j, m)] for m in range(M_PER_BAND))
        for b, j in CHUNK_LIST
    }


MODES = _gen_modes()
DELAY = 8
TAIL_FAST = 8  # last iterations flush with DELAY=1

_cached = None


def _build():
    import concourse.bacc as bacc
    import concourse.tile as tile
    from concourse import mybir

    first_nonA = {}
    last_nonA = {}
    for b, j, m in ITER_ORDER:
        if MODES[(b, j)][m] != "A":
            first_nonA.setdefault((b, m), j)
            last_nonA[(b, m)] = j
    for b, _, _, nj in BANDS:
        for m in range(M_PER_BAND):
            assert (b, m) in first_nonA, f"(band={b}, m={m}) all A"

    # acc adds mostly on Pool; every 8th D chunk's add goes to DVE
    add_on_dve = set()
    di = 0
    for b, j, m in ITER_ORDER:
        if MODES[(b, j)][m] != "A":
            if di % 8 == 7:
                add_on_dve.add((b, j, m))
            di += 1

    # colsum group + column-group base per (band, j)
    cs_group = {}
    cs_base = {}
    cs_last_j = {}
    for gi, (b, js) in enumerate(COL_GROUPS):
        for sub, j in enumerate(js):
            cs_group[(b, j)] = gi
            cs_base[(b, j)] = 64 * sub
        cs_last_j[gi] = js[-1]

    n_iters = len(ITER_ORDER)

    nc = bacc.Bacc(
        "TRN2",
        target_bir_lowering=False,
        debug=False,
        num_devices=NCORES,
    )
    f32 = mybir.dt.float32
    bf16 = mybir.dt.bfloat16
    u16 = mybir.dt.uint16
    fp8 = mybir.dt.float8e4

    zT_dram = nc.dram_tensor("zT", (64, 2, N), fp8, kind="ExternalInput")
    cneg_dram = nc.dram_tensor("c_neg", (128, 2), f32, kind="ExternalInput")
    bsch_dram = nc.dram_tensor("b_sch", (128, 2), f32, kind="ExternalInput")
    s_dram = nc.dram_tensor("s_out", (128, 16), f32, kind="ExternalOutput")
    pos_dram = nc.dram_tensor("pos_out", (128, M_PER_BAND), f32, kind="ExternalOutput")
    col_dram = nc.dram_tensor(
        "col_out", (N_COLCHUNKS, 128, 512), f32, kind="ExternalOutput"
    )

    id_np = np.eye(128, dtype=np.float32)
    id_dram = nc.inline_tensor(id_np, name="id_mask")
    neg_np = np.zeros((128, 128), dtype=np.float32)
    np.fill_diagonal(neg_np, -1.0e30)
    neg_dram = nc.inline_tensor(neg_np, name="neg_mask")
    ones_dram = nc.inline_tensor(
        np.ones((128, 1), dtype=ml_dtypes.bfloat16), name="ones_vec"
    )

    with tile.TileContext(nc) as tc:
        with (
            tc.tile_pool(name="zbuf", bufs=1) as zpool,
            tc.tile_pool(name="consts", bufs=1) as cpool,
            tc.tile_pool(name="persist", bufs=1) as perpool,
            tc.tile_pool(name="psum", bufs=3, space="PSUM") as ppool,
            tc.tile_pool(name="colsum", bufs=2, space="PSUM") as cspool,
            tc.tile_pool(name="expout", bufs=12) as epool,
            tc.tile_pool(name="tmp", bufs=3) as tpool,
        ):
            z_sb = zpool.tile([64, 2, N], fp8)
            # exp ACT-table load first — needs no DMA'd inputs
            warm_in = cpool.tile([128, 1], f32)
            nc.gpsimd.memset(warm_in, 0.0)
            warm = cpool.tile([128, 1], f32)
            nc.scalar.activation(
                out=warm,
                in_=warm_in,
                func=mybir.ActivationFunctionType.Exp,
                bias=0.0,
                scale=0.0,
            )
            # two parallel queues for the first two slices the pipeline needs
            nc.sync.dma_start(
                out=z_sb[:, :, 0:CHUNK], in_=zT_dram[:, :, 0:CHUNK]
            )
            nc.gpsimd.dma_start(
                out=z_sb[:, :, CHUNK : 2 * CHUNK],
                in_=zT_dram[:, :, CHUNK : 2 * CHUNK],
            )
            cneg_sb = cpool.tile([128, 2], f32)
            nc.sync.dma_start(out=cneg_sb, in_=cneg_dram[:, :])
            bsch_sb = cpool.tile([128, 2], f32)
            nc.sync.dma_start(out=bsch_sb, in_=bsch_dram[:, :])
            ones_sb = cpool.tile([128, 1], bf16)
            nc.sync.dma_start(out=ones_sb, in_=ones_dram[:, :])
            negm = cpool.tile([128, 128], f32)
            nc.gpsimd.dma_start(out=negm, in_=neg_dram[:, :])
            idm = cpool.tile([128, 128], f32)
            nc.sync.dma_start(out=idm, in_=id_dram[:, :])

            cuts = [2 * CHUNK, 6144, 10752, N]
            for qs, qe in zip(cuts, cuts[1:]):
                nc.sync.dma_start(
                    out=z_sb[:, :, qs:qe], in_=zT_dram[:, :, qs:qe]
                )

            s_band = perpool.tile([128, 16, 9], f32)
            nc.gpsimd.memset(s_band, 0.0)
            acc = perpool.tile([128, 16, CHUNK], bf16)
            acc_red = perpool.tile([128, 16], f32)
            s_red = perpool.tile([128, 16], f32)
            s_out_sb = perpool.tile([128, 16], f32)
            pos_sb = perpool.tile([128, M_PER_BAND], f32)

            pending = []       # colsum / acc / final emissions
            pending_dve = []   # DVE-heavy deferred ops (reduces): 1 per iter

            def flush(now, dve_budget=1):
                rest = []
                for due, fn in pending:
                    if due <= now:
                        fn()
                    else:
                        rest.append((due, fn))
                pending[:] = rest
                n = 0
                rest = []
                for due, fn in pending_dve:
                    if due <= now and n < dve_budget:
                        fn()
                        n += 1
                    else:
                        rest.append((due, fn))
                pending_dve[:] = rest

            def emit_bandA_final():
                nc.vector.tensor_reduce(
                    out=s_red[:, 0:8],
                    in_=s_band[:, 0:8, :],
                    axis=mybir.AxisListType.X,
                    op=mybir.AluOpType.add,
                )
                nc.vector.tensor_add(
                    s_out_sb[:, 0:8], s_red[:, 0:8], acc_red[:, 0:8]
                )
                nc.sync.dma_start(out=s_dram[:, 0:8], in_=s_out_sb[:, 0:8])
                nc.sync.dma_start(out=pos_dram[:, :], in_=pos_sb)

            it = 0
            for band, row_base, col_base, n_chunks in BANDS:
                if band == 1:
                    pending.append((it + DELAY + 2, emit_bandA_final))
                colsums = {}
                for j, m in _band_iter_order(n_chunks):
                    mode = MODES[(band, j)][m]
                    cbase = col_base + j * CHUNK
                    r0 = row_base + m * 128
                    k16 = band * 8 + m
                    delay = 1 if it >= n_iters - TAIL_FAST else DELAY
                    colsum = None
                    gi = cg = None
                    if j != 0:
                        gi = cs_group[(band, j)]
                        cg = cs_base[(band, j)]
                        if gi not in colsums:
                            colsums[gi] = cspool.tile(
                                [128, 512], f32, tag="cs", name=f"cs_{gi}"
                            )
                        colsum = colsums[gi]
                    ps = ppool.tile([128, CHUNK], f32, tag="ps")
                    for k in range(NSLOTS):
                        nc.tensor.matmul(
                            ps[:, k * 512 : (k + 1) * 512],
                            lhsT=z_sb[:, :, r0 : r0 + 128],
                            rhs=z_sb[
                                :, :, cbase + k * 512 : cbase + (k + 1) * 512
                            ],
                            start=True,
                            stop=True,
                            perf_mode=mybir.MatmulPerfMode.DoubleRow,
                        )
                    assert not (j == 0 and mode == "A")
                    if j == 0:
                        # mask self-similarity diagonal (heavy rows would
                        # wrap the Schraudolph bits into large negatives)
                        nc.vector.tensor_add(
                            ps[:, m * 128 : m * 128 + 128],
                            ps[:, m * 128 : m * 128 + 128],
                            negm,
                        )
                    if band == 0 and j == 8:
                        tmp = tpool.tile([128, 128], f32, tag="pos")
                        nc.vector.tensor_mul(
                            tmp, ps[:, m * 128 : m * 128 + 128], idm
                        )
                        nc.vector.tensor_reduce(
                            out=pos_sb[:, m : m + 1],
                            in_=tmp,
                            axis=mybir.AxisListType.X,
                            op=mybir.AluOpType.add,
                        )
                    et = epool.tile(
                        [128, CHUNK], bf16, tag="exp", name=f"et_{band}_{j}_{m}"
                    )
                    if mode == "A":
                        nc.scalar.activation(
                            out=et,
                            in_=ps,
                            func=mybir.ActivationFunctionType.Exp,
                            bias=cneg_sb[:, band : band + 1],
                            scale=1.0,
                            accum_out=s_band[:, k16, j : j + 1],
                        )
                    else:
                        nc.vector.tensor_scalar(
                            out=et.bitcast(u16),
                            in0=ps,
                            scalar1=SCH_SCALE,
                            scalar2=bsch_sb[:, band : band + 1],
                            op0=mybir.AluOpType.mult,
                            op1=mybir.AluOpType.add,
                        )

                        def emit_acc(et=et, k16=k16, band=band, m=m, j=j):
                            eng = (
                                nc.vector
                                if (band, j, m) in add_on_dve
                                else nc.gpsimd
                            )
                            if first_nonA[(band, m)] == j:
                                eng.tensor_copy(acc[:, k16, :], et)
                            else:
                                eng.tensor_add(acc[:, k16, :], acc[:, k16, :], et)

                        pending.append((it + delay, emit_acc))
                        if last_nonA[(band, m)] == j:
                            def emit_red(k16=k16):
                                nc.vector.tensor_reduce(
                                    out=acc_red[:, k16 : k16 + 1],
                                    in_=acc[:, k16, :],
                                    axis=mybir.AxisListType.X,
                                    op=mybir.AluOpType.add,
                                )

                            pending_dve.append((it + delay + 1, emit_red))

                    if j != 0:
                        def emit_cs(et=et, colsum=colsum, m=m, cg=cg):
                            for s in range(NSLOTS):
                                nc.tensor.matmul(
                                    colsum[cg + 32 * s : cg + 32 * s + 1, :],
                                    lhsT=ones_sb,
                                    rhs=et[:, s * 512 : (s + 1) * 512],
                                    start=(m == 0),
                                    stop=(m == M_PER_BAND - 1),
                                    tile_position=(0, cg + 32 * s),
                                )

                        pending.append((it + delay, emit_cs))

                        if m == M_PER_BAND - 1 and cs_last_j[gi] == j:
                            def emit_stage(colsum=colsum, gi=gi):
                                stage = tpool.tile(
                                    [128, 512], f32, tag="stage"
                                )
                                nc.scalar.copy(stage, colsum)
                                nc.sync.dma_start(
                                    out=col_dram[gi, :, :], in_=stage
                                )

                            pending.append((it + delay + 1, emit_stage))

                    it += 1
                    flush(it)

            flush(it + DELAY + 10, dve_budget=99)

            nc.vector.tensor_reduce(
                out=s_red[:, 8:16],
                in_=s_band[:, 8:16, :],
                axis=mybir.AxisListType.X,
                op=mybir.AluOpType.add,
            )
            nc.vector.tensor_add(
                s_out_sb[:, 8:16], s_red[:, 8:16], acc_red[:, 8:16]
            )
            nc.sync.dma_start(out=s_dram[:, 8:16], in_=s_out_sb[:, 8:16])

    nc.compile()
    return nc


def _get_nc():
    global _cached
    if _cached is None:
        _cached = _build()
    return _cached


def _prep(z_i: np.ndarray, z_j: np.ndarray):
    z = np.concatenate(
        [np.asarray(z_i, np.float32), np.asarray(z_j, np.float32)], axis=0
    )
    w = z * np.float32(math.sqrt(2.0))
    wnorm = np.linalg.norm(w.astype(np.float64), axis=1)
    c_band = np.array(
        [
            ROWMAX_COEF * np.median(wnorm[b * KB : (b + 1) * KB]) + C_SHIFT
            for b in range(16)
        ],
        dtype=np.float64,
    )
    in_maps = []
    for c in range(NCORES):
        wc = np.roll(w, -c * KB, axis=0)
        zT = np.ascontiguousarray(
            wc.T.reshape(2, 64, N).transpose(1, 0, 2)
        ).astype(ml_dtypes.float8_e4m3)
        cneg = np.zeros((128, 2), dtype=np.float32)
        cneg[:, 0] = -c_band[c]
        cneg[:, 1] = -c_band[c + 8]
        bsch = np.zeros((128, 2), dtype=np.float32)
        bsch[:, 0] = 16256.0 - c_band[c] * SCH_SCALE + SCH_ADJ
        bsch[:, 1] = 16256.0 - c_band[c + 8] * SCH_SCALE + SCH_ADJ
        in_maps.append({"zT": zT, "c_neg": cneg, "b_sch": bsch})
    return w, c_band, in_maps


def _finish(w, c_band, results):
    s_abs = np.zeros(N, dtype=np.float64)
    pos = np.zeros(N, dtype=np.float64)
    for c in range(NCORES):
        r = results[c]
        s_dev = r["s_out"].astype(np.float64)
        pos_dev = r["pos_out"].astype(np.float64)
        col_dev = r["col_out"].astype(np.float64)
        for band, kb in ((0, c), (1, c + 8)):
            scale = math.exp(c_band[kb])
            rows = s_dev[:, band * 8 : band * 8 + 8].T.reshape(KB)
            g0 = kb * KB
            s_abs[g0 : g0 + KB] += rows * scale
        for gi, (band, js) in enumerate(COL_GROUPS):
            kb = c if band == 0 else c + 8
            scale = math.exp(c_band[kb])
            for sub, j in enumerate(js):
                for sl in range(NSLOTS):
                    L = band * 8192 + j * CHUNK + sl * 512
                    vals = col_dev[gi, 64 * sub + 32 * sl, :] * scale
                    g = (c * KB + L) % N
                    s_abs[g : g + 512] += vals
        p_rows = pos_dev.T.reshape(KB)
        pos[c * KB : c * KB + KB] = p_rows
        pos[c * KB + 8192 : c * KB + 8192 + KB] = p_rows

    with np.errstate(divide="ignore", invalid="ignore"):
        lse = np.log(s_abs)
    bad = ~np.isfinite(lse)
    if bad.any():
        idx = np.nonzero(bad)[0]
        wb = w[idx].astype(np.float64)
        sim_b = wb @ w.astype(np.float64).T
        for ii, rr in enumerate(idx):
            sim_b[ii, rr] = -np.inf
        m_b = sim_b.max(axis=1)
        lse[idx] = np.log(np.exp(sim_b - m_b[:, None]).sum(axis=1)) + m_b
        pos_idx = np.where(idx < TRAIN_NUM, idx + TRAIN_NUM, idx - TRAIN_NUM)
        pos[idx] = np.einsum("ij,ij->i", wb, w[pos_idx].astype(np.float64))
    loss = (lse - pos).mean()
    return np.float32(loss)


def run(z_i, z_j, trace=False, **kw):
    from concourse.bass_utils import run_bass_kernel_spmd

    nc = _get_nc()
    w, c_band, in_maps = _prep(z_i, z_j)
    res = run_bass_kernel_spmd(
        nc, in_maps, core_ids=list(range(NCORES)), trace=trace, **kw
    )
    return _finish(w, c_band, res.results), res


def kernel(z_i, z_j):
    loss, _ = run(z_i, z_j, trace=False)
    return loss


# revision 6
# speedup vs baseline: 1.1199x; 1.0387x over previous
"""NT-Xent instance loss on 8 Trainium2 NeuronCores, V4.

Symmetric 17-kiloblock decomposition: z (16384x128) is split into 16
kilobands; core c owns bands c and c+8 and computes each off-diagonal sim
kiloblock exactly once (17 column chunks of 1024 per core), contributing
row sums directly and column sums (the transposed block's row sums) via
ones-vector matmuls.

PSUM readout — the bottleneck; only ACT and DVE can read PSUM:
  - mode 'A' (ACT): exact exp activation, bf16 out, fp32 accum_out row
    sums into s_band.
  - mode 'D' (DVE): Schraudolph exp — uint16(sim*S + B) bitcast to bf16
    approximates exp(sim - C) to ~1.6% (the fp32->uint16 convert
    saturates negatives to 0 = bf16 +0.0; verified on HW). Row sums via
    bf16 accumulators: tensor_add on the otherwise-idle Pool engine
    (an SBUF-only engine), per-(band,m) fp32 tensor_reduce on DVE.

j==0 (the D=0 kiloblock, colsums discarded by the host) is all-D with a
-1e30 diagonal mask add on DVE (local, cheap); exact exp of the unmasked
diagonal would be inf, and unmasked Schraudolph bits can wrap into
large-negative bf16 for heavy rows.

Colsum PSUM: one [128,512] bank serves TWO j-chunks via tile_position
column groups (slots at partitions 0/32 and 64/96), so only one stage
copy + DMA per j-pair. Emission is software-pipelined: colsum matmuls
and accumulator adds trail their producer by DELAY m-iterations, the 8
colsum-free j==0 iterations are spliced between other chunks, and the
tail iterations flush fast so the drain stays short.
"""

import math

import numpy as np
import ml_dtypes

TRAIN_NUM = 8192
EMBED = 128
N = 2 * TRAIN_NUM
NCORES = 8
KB = 1024
M_PER_BAND = KB // 128
CHUNK = 1024                 # PSUM sim chunk (2 banks); 2 col-sum slots
NSLOTS = CHUNK // 512
ROWMAX_COEF = math.sqrt(2.0 * math.log(N - 1)) * math.sqrt(2.0)
C_SHIFT = 6.0

SCH_SCALE = 128.0 / math.log(2.0)
SCH_ADJ = -7.5

BANDS = [
    (0, 0, 0, 9),            # (band, row_base, col_base, n_chunks)
    (1, 8192, 8192, 8),
]
CHUNK_LIST = [(b, j) for b, _, _, nj in BANDS for j in range(nj)]
# colsum groups: pairs of j>=1 chunks sharing one PSUM bank
COL_GROUPS = []
for _b, _, _, _nj in BANDS:
    _js = [j for j in range(1, _nj)]
    for _i in range(0, len(_js), 2):
        COL_GROUPS.append((_b, tuple(_js[_i : _i + 2])))
N_COLCHUNKS = len(COL_GROUPS)  # 8 groups of up to 2 chunks


def _band_iter_order(n_chunks):
    """j-major for j>=1 with the 8 colsum-free j==0 iterations spliced in
    every 3rd slot near the front."""
    base = [(j, m) for j in range(1, n_chunks) for m in range(M_PER_BAND)]
    order = []
    k = 0
    for m in range(M_PER_BAND):
        order.extend(base[k : k + 2])
        k += 2
        order.append((0, m))
    order.extend(base[k:])
    return order


ITER_ORDER = []
for _b, _, _, _nj in BANDS:
    ITER_ORDER.extend((_b, j, m) for j, m in _band_iter_order(_nj))


def _gen_modes():
    """j0 all-D (diagonal masked there); elsewhere A/D Bresenham-spread.
    A84 / D52 overall."""
    n = len(ITER_ORDER)
    target = {"A": 82, "D": 54}
    credit = {e: 0.0 for e in "AD"}
    left = dict(target)
    assign = {}
    prev = ""
    for i, (b, j, m) in enumerate(ITER_ORDER):
        for e in "AD":
            credit[e] += left[e] / (n - i)
        cands = [e for e in "AD" if left[e] > 0 and not (e == "A" and j == 0)]
        e = max(cands, key=lambda x: credit[x])
        assign[(b, j, m)] = e
        credit[e] -= 1.0
        left[e] -= 1
    # re-lay the last 16 positions so A's alternate with D's (the adaptive
    # spread tends to leave an A-convoy at the very end, stalling PE there)
    tail_keys = ITER_ORDER[n - 16 :]
    tail_modes = [assign[k] for k in tail_keys]
    n_a = tail_modes.count("A")
    n_d = len(tail_modes) - n_a
    lay = []
    a_left, d_left = n_a, n_d
    take_a = True
    for _ in range(len(tail_modes)):
        if (take_a and a_left > 0) or d_left == 0:
            lay.append("A"); a_left -= 1
        else:
            lay.append("D"); d_left -= 1
        take_a = not take_a
    for k, e in zip(tail_keys, lay):
        assign[k] = e
    return {
        (b, j): "".join(assign[(b, j, m)] for m in range(M_PER_BAND))
        for b, j in CHUNK_LIST
    }


MODES = _gen_modes()
DELAY = 16
TAIL_FAST = 0  # last iterations flush with DELAY=1

_cached = None


def _build():
    import concourse.bacc as bacc
    import concourse.tile as tile
    from concourse import mybir

    first_nonA = {}
    last_nonA = {}
    for b, j, m in ITER_ORDER:
        if MODES[(b, j)][m] != "A":
            first_nonA.setdefault((b, m), j)
            last_nonA[(b, m)] = j
    for b, _, _, nj in BANDS:
        for m in range(M_PER_BAND):
            assert (b, m) in first_nonA, f"(band={b}, m={m}) all A"

    # acc adds mostly on Pool; every 8th D chunk's add goes to DVE
    add_on_dve = set()
    di = 0
    for b, j, m in ITER_ORDER:
        if MODES[(b, j)][m] != "A":
            if di % 8 == 7:
                add_on_dve.add((b, j, m))
            di += 1

    # colsum group + column-group base per (band, j)
    cs_group = {}
    cs_base = {}
    cs_last_j = {}
    for gi, (b, js) in enumerate(COL_GROUPS):
        for sub, j in enumerate(js):
            cs_group[(b, j)] = gi
            cs_base[(b, j)] = 64 * sub
        cs_last_j[gi] = js[-1]

    n_iters = len(ITER_ORDER)

    nc = bacc.Bacc(
        "TRN2",
        target_bir_lowering=False,
        debug=False,
        num_devices=NCORES,
    )
    f32 = mybir.dt.float32
    bf16 = mybir.dt.bfloat16
    u16 = mybir.dt.uint16
    fp8 = mybir.dt.float8e4

    zT_dram = nc.dram_tensor("zT", (64, 2, N), fp8, kind="ExternalInput")
    cneg_dram = nc.dram_tensor("c_neg", (128, 2), f32, kind="ExternalInput")
    bsch_dram = nc.dram_tensor("b_sch", (128, 2), f32, kind="ExternalInput")
    s_dram = nc.dram_tensor("s_out", (128, 16), f32, kind="ExternalOutput")
    pos_dram = nc.dram_tensor("pos_out", (128, M_PER_BAND), f32, kind="ExternalOutput")
    col_dram = nc.dram_tensor(
        "col_out", (N_COLCHUNKS, 128, 512), f32, kind="ExternalOutput"
    )

    id_np = np.eye(128, dtype=np.float32)
    id_dram = nc.inline_tensor(id_np, name="id_mask")
    neg_np = np.zeros((128, 128), dtype=np.float32)
    np.fill_diagonal(neg_np, -1.0e30)
    neg_dram = nc.inline_tensor(neg_np, name="neg_mask")
    ones_dram = nc.inline_tensor(
        np.ones((128, 1), dtype=ml_dtypes.bfloat16), name="ones_vec"
    )

    with tile.TileContext(nc) as tc:
        with (
            tc.tile_pool(name="zbuf", bufs=1) as zpool,
            tc.tile_pool(name="consts", bufs=1) as cpool,
            tc.tile_pool(name="persist", bufs=1) as perpool,
            tc.tile_pool(name="psum", bufs=3, space="PSUM") as ppool,
            tc.tile_pool(name="colsum", bufs=2, space="PSUM") as cspool,
            tc.tile_pool(name="expout", bufs=26) as epool,
            tc.tile_pool(name="tmp", bufs=3) as tpool,
        ):
            z_sb = zpool.tile([64, 2, N], fp8)
            # exp ACT-table load first — needs no DMA'd inputs
            warm_in = cpool.tile([128, 1], f32)
            nc.gpsimd.memset(warm_in, 0.0)
            warm = cpool.tile([128, 1], f32)
            nc.scalar.activation(
                out=warm,
                in_=warm_in,
                func=mybir.ActivationFunctionType.Exp,
                bias=0.0,
                scale=0.0,
            )
            # two parallel queues for the first two slices the pipeline needs
            nc.sync.dma_start(
                out=z_sb[:, :, 0:CHUNK], in_=zT_dram[:, :, 0:CHUNK]
            )
            nc.gpsimd.dma_start(
                out=z_sb[:, :, CHUNK : 2 * CHUNK],
                in_=zT_dram[:, :, CHUNK : 2 * CHUNK],
            )
            cneg_sb = cpool.tile([128, 2], f32)
            nc.sync.dma_start(out=cneg_sb, in_=cneg_dram[:, :])
            bsch_sb = cpool.tile([128, 2], f32)
            nc.sync.dma_start(out=bsch_sb, in_=bsch_dram[:, :])
            ones_sb = cpool.tile([128, 1], bf16)
            nc.sync.dma_start(out=ones_sb, in_=ones_dram[:, :])
            negm = cpool.tile([128, 128], f32)
            nc.gpsimd.dma_start(out=negm, in_=neg_dram[:, :])
            idm = cpool.tile([128, 128], f32)
            nc.sync.dma_start(out=idm, in_=id_dram[:, :])

            cuts = [2 * CHUNK, 6144, 10752, N]
            for qs, qe in zip(cuts, cuts[1:]):
                nc.sync.dma_start(
                    out=z_sb[:, :, qs:qe], in_=zT_dram[:, :, qs:qe]
                )

            s_band = perpool.tile([128, 16, 9], f32)
            nc.gpsimd.memset(s_band, 0.0)
            acc = perpool.tile([128, 16, CHUNK], bf16)
            acc_red = perpool.tile([128, 16], f32)
            s_red = perpool.tile([128, 16], f32)
            s_out_sb = perpool.tile([128, 16], f32)
            pos_sb = perpool.tile([128, M_PER_BAND], f32)

            pending = []       # colsum / acc / final emissions
            pending_dve = []   # DVE-heavy deferred ops (reduces): 1 per iter

            def flush(now, dve_budget=1):
                rest = []
                for due, fn in pending:
                    if due <= now:
                        fn()
                    else:
                        rest.append((due, fn))
                pending[:] = rest
                n = 0
                rest = []
                for due, fn in pending_dve:
                    if due <= now and n < dve_budget:
                        fn()
                        n += 1
                    else:
                        rest.append((due, fn))
                pending_dve[:] = rest

            def emit_bandA_final():
                nc.vector.tensor_reduce(
                    out=s_red[:, 0:8],
                    in_=s_band[:, 0:8, :],
                    axis=mybir.AxisListType.X,
                    op=mybir.AluOpType.add,
                )
                nc.vector.tensor_add(
                    s_out_sb[:, 0:8], s_red[:, 0:8], acc_red[:, 0:8]
                )
                nc.sync.dma_start(out=s_dram[:, 0:8], in_=s_out_sb[:, 0:8])
                nc.sync.dma_start(out=pos_dram[:, :], in_=pos_sb)

            it = 0
            for band, row_base, col_base, n_chunks in BANDS:
                if band == 1:
                    pending.append((it + DELAY + 2, emit_bandA_final))
                colsums = {}
                for j, m in _band_iter_order(n_chunks):
                    mode = MODES[(band, j)][m]
                    cbase = col_base + j * CHUNK
                    r0 = row_base + m * 128
                    k16 = band * 8 + m
                    delay = 1 if it >= n_iters - TAIL_FAST else DELAY
                    colsum = None
                    gi = cg = None
                    if j != 0:
                        gi = cs_group[(band, j)]
                        cg = cs_base[(band, j)]
                        if gi not in colsums:
                            colsums[gi] = cspool.tile(
                                [128, 512], f32, tag="cs", name=f"cs_{gi}"
                            )
                        colsum = colsums[gi]
                    ps = ppool.tile([128, CHUNK], f32, tag="ps")
                    for k in range(NSLOTS):
                        nc.tensor.matmul(
                            ps[:, k * 512 : (k + 1) * 512],
                            lhsT=z_sb[:, :, r0 : r0 + 128],
                            rhs=z_sb[
                                :, :, cbase + k * 512 : cbase + (k + 1) * 512
                            ],
                            start=True,
                            stop=True,
                            perf_mode=mybir.MatmulPerfMode.DoubleRow,
                        )
                    assert not (j == 0 and mode == "A")
                    if j == 0:
                        # mask self-similarity diagonal (heavy rows would
                        # wrap the Schraudolph bits into large negatives)
                        nc.vector.tensor_add(
                            ps[:, m * 128 : m * 128 + 128],
                            ps[:, m * 128 : m * 128 + 128],
                            negm,
                        )
                    if band == 0 and j == 8:
                        tmp = tpool.tile([128, 128], f32, tag="pos")
                        nc.vector.tensor_mul(
                            tmp, ps[:, m * 128 : m * 128 + 128], idm
                        )
                        nc.vector.tensor_reduce(
                            out=pos_sb[:, m : m + 1],
                            in_=tmp,
                            axis=mybir.AxisListType.X,
                            op=mybir.AluOpType.add,
                        )
                    et = epool.tile(
                        [128, CHUNK], bf16, tag="exp", name=f"et_{band}_{j}_{m}"
                    )
                    if mode == "A":
                        nc.scalar.activation(
                            out=et,
                            in_=ps,
                            func=mybir.ActivationFunctionType.Exp,
                            bias=cneg_sb[:, band : band + 1],
                            scale=1.0,
                            accum_out=s_band[:, k16, j : j + 1],
                        )
                    else:
                        nc.vector.tensor_scalar(
                            out=et.bitcast(u16),
                            in0=ps,
                            scalar1=SCH_SCALE,
                            scalar2=bsch_sb[:, band : band + 1],
                            op0=mybir.AluOpType.mult,
                            op1=mybir.AluOpType.add,
                        )

                        def emit_acc(et=et, k16=k16, band=band, m=m, j=j):
                            eng = (
                                nc.vector
                                if (band, j, m) in add_on_dve
                                else nc.gpsimd
                            )
                            if first_nonA[(band, m)] == j:
                                eng.tensor_copy(acc[:, k16, :], et)
                            else:
                                eng.tensor_add(acc[:, k16, :], acc[:, k16, :], et)

                        pending.append((it + delay, emit_acc))
                        if last_nonA[(band, m)] == j:
                            def emit_red(k16=k16):
                                nc.vector.tensor_reduce(
                                    out=acc_red[:, k16 : k16 + 1],
                                    in_=acc[:, k16, :],
                                    axis=mybir.AxisListType.X,
                                    op=mybir.AluOpType.add,
                                )

                            pending_dve.append((it + delay + 1, emit_red))

                    if j != 0:
                        def emit_cs(et=et, colsum=colsum, m=m, cg=cg):
                            for s in range(NSLOTS):
                                nc.tensor.matmul(
                                    colsum[cg + 32 * s : cg + 32 * s + 1, :],
                                    lhsT=ones_sb,
                                    rhs=et[:, s * 512 : (s + 1) * 512],
                                    start=(m == 0),
                                    stop=(m == M_PER_BAND - 1),
                                    tile_position=(0, cg + 32 * s),
                                )

                        pending.append((it + delay, emit_cs))

                        if m == M_PER_BAND - 1 and cs_last_j[gi] == j:
                            def emit_stage(colsum=colsum, gi=gi):
                                stage = tpool.tile(
                                    [128, 512], f32, tag="stage"
                                )
                                nc.scalar.copy(stage, colsum)
                                nc.sync.dma_start(
                                    out=col_dram[gi, :, :], in_=stage
                                )

                            pending.append((it + delay + 1, emit_stage))

                    it += 1
                    flush(it)

            flush(it + DELAY + 10, dve_budget=99)

            nc.vector.tensor_reduce(
                out=s_red[:, 8:16],
                in_=s_band[:, 8:16, :],
                axis=mybir.AxisListType.X,
                op=mybir.AluOpType.add,
            )
            nc.vector.tensor_add(
                s_out_sb[:, 8:16], s_red[:, 8:16], acc_red[:, 8:16]
            )
            nc.sync.dma_start(out=s_dram[:, 8:16], in_=s_out_sb[:, 8:16])

    nc.compile()
    return nc


def _get_nc():
    global _cached
    if _cached is None:
        _cached = _build()
    return _cached


def _prep(z_i: np.ndarray, z_j: np.ndarray):
    z = np.concatenate(
        [np.asarray(z_i, np.float32), np.asarray(z_j, np.float32)], axis=0
    )
    w = z * np.float32(math.sqrt(2.0))
    wnorm = np.linalg.norm(w.astype(np.float64), axis=1)
    c_band = np.array(
        [
            ROWMAX_COEF * np.median(wnorm[b * KB : (b + 1) * KB]) + C_SHIFT
            for b in range(16)
        ],
        dtype=np.float64,
    )
    in_maps = []
    for c in range(NCORES):
        wc = np.roll(w, -c * KB, axis=0)
        zT = np.ascontiguousarray(
            wc.T.reshape(2, 64, N).transpose(1, 0, 2)
        ).astype(ml_dtypes.float8_e4m3)
        cneg = np.zeros((128, 2), dtype=np.float32)
        cneg[:, 0] = -c_band[c]
        cneg[:, 1] = -c_band[c + 8]
        bsch = np.zeros((128, 2), dtype=np.float32)
        bsch[:, 0] = 16256.0 - c_band[c] * SCH_SCALE + SCH_ADJ
        bsch[:, 1] = 16256.0 - c_band[c + 8] * SCH_SCALE + SCH_ADJ
        in_maps.append({"zT": zT, "c_neg": cneg, "b_sch": bsch})
    return w, c_band, in_maps


def _finish(w, c_band, results):
    s_abs = np.zeros(N, dtype=np.float64)
    pos = np.zeros(N, dtype=np.float64)
    for c in range(NCORES):
        r = results[c]
        s_dev = r["s_out"].astype(np.float64)
        pos_dev = r["pos_out"].astype(np.float64)
        col_dev = r["col_out"].astype(np.float64)
        for band, kb in ((0, c), (1, c + 8)):
            scale = math.exp(c_band[kb])
            rows = s_dev[:, band * 8 : band * 8 + 8].T.reshape(KB)
            g0 = kb * KB
            s_abs[g0 : g0 + KB] += rows * scale
        for gi, (band, js) in enumerate(COL_GROUPS):
            kb = c if band == 0 else c + 8
            scale = math.exp(c_band[kb])
            for sub, j in enumerate(js):
                for sl in range(NSLOTS):
                    L = band * 8192 + j * CHUNK + sl * 512
                    vals = col_dev[gi, 64 * sub + 32 * sl, :] * scale
                    g = (c * KB + L) % N
                    s_abs[g : g + 512] += vals
        p_rows = pos_dev.T.reshape(KB)
        pos[c * KB : c * KB + KB] = p_rows
        pos[c * KB + 8192 : c * KB + 8192 + KB] = p_rows

    with np.errstate(divide="ignore", invalid="ignore"):
        lse = np.log(s_abs)
    bad = ~np.isfinite(lse)
    if bad.any():
        idx = np.nonzero(bad)[0]
        wb = w[idx].astype(np.float64)
        sim_b = wb @ w.astype(np.float64).T
        for ii, rr in enumerate(idx):
            sim_b[ii, rr] = -np.inf
        m_b = sim_b.max(axis=1)
        lse[idx] = np.log(np.exp(sim_b - m_b[:, None]).sum(axis=1)) + m_b
        pos_idx = np.where(idx < TRAIN_NUM, idx + TRAIN_NUM, idx - TRAIN_NUM)
        pos[idx] = np.einsum("ij,ij->i", wb, w[pos_idx].astype(np.float64))
    loss = (lse - pos).mean()
    return np.float32(loss)


def run(z_i, z_j, trace=False, **kw):
    from concourse.bass_utils import run_bass_kernel_spmd

    nc = _get_nc()
    w, c_band, in_maps = _prep(z_i, z_j)
    res = run_bass_kernel_spmd(
        nc, in_maps, core_ids=list(range(NCORES)), trace=trace, **kw
    )
    return _finish(w, c_band, res.results), res


def kernel(z_i, z_j):
    loss, _ = run(z_i, z_j, trace=False)
    return loss
